# revision 13
# baseline (speedup 1.0000x reference)
"""Two-layer GraphConv (DGL norm='none') on 8 Trainium2 NeuronCores.

Math (per layer):  out = relu( segment_sum((x W)[src] by dst) + b )
Projection commutes with the sum, so we compute
                   out = relu( segment_sum(x[src] by dst) @ W + b )

Sharding: 1D partition of dst nodes across 8 cores (12544 each, 12500
real). Each core gathers the source rows of its incident edges with
`dma_gather` (one 256B descriptor per edge) and aggregates them ON-CHIP
with the tensor engine: for each 128-edge message tile and each 256-dst
window it overlaps, a one-hot routing matrix S (S[i,j] = 1 iff edge i's
dst is window column j) is built on the vector engine with an
iota/is_equal compare, and psum_w += msg_tile^T @ S accumulates the
feature-major aggregate. This removes the baseline's dma_scatter_add
path entirely (the scatter descriptors were ~45% of the Pool-engine
critical path, and the round/budget machinery they required padded the
gather stream by ~8%).

Slot layout: edges are bucketed per (block, chunk) where a block is 4
consecutive 256-dst windows (psum-bank limited) and a chunk is one of 4
gather-table chunks (dma_gather indices are int16, so the 100352-row
table is read in 4 chunks of 25088 rows). Within a bucket edges are
sorted by dst; buckets are padded to a multiple of 128 (pad slots
gather row 0 with label -1, which matches no window column and so
contributes nothing). Buckets are uniform across cores/layers (max
over all instances) so the SPMD program is fixed; the per-tile window
lists are the union over all instances (a matmul for a window with no
edges in the tile adds zero).

Perf notes (all hardware-measured):
  * gather calls are 768 slots and rotate over all 4 SWDGE queues
    (queue q is served by Q7 core pair 2q/2q+1 and its own descriptor
    ring); this cut the per-slot gather cost from ~9.0ns to ~6.3ns by
    eliminating ring-drain stalls.
  * messages are copied f32->f16 on the otherwise-idle scalar engine
    and S is built in f16 (labels stay f32, required by is_equal, and
    fp16 keeps integers up to 2048 exact where bf16 would alias
    256/257), halving the PE matmul cost (fp16 is full rate, fp32 is
    4 cycles/row).
  * phase B (projection, relu+bias, transpose to node-major, hout DMA)
    is fused per window into the aggregation stream, shrinking the
    serial tail after the last gather from ~110us to ~15us.
  * engine balance per ~1.15ms layer: GpSimd (gather desc-gen) 1045us,
    DVE (is_equal S-builds) 1096us, PE 720us, Scalar 165us.

The layer-1 -> layer-2 exchange (every core needs all h rows) is done
by the host between the two dispatches of the same NEFF; layer 1 needs
no exchange because the full x is resident on every core.
"""

import sys
from dataclasses import dataclass, replace

import numpy as np

sys.path.insert(0, "/opt/trn_rl_repo")


@dataclass(frozen=True)
class Cfg:
    n_nodes: int = 100000
    feat: int = 64
    ncores: int = 8
    shard: int = 12500       # real dst rows per core
    shard_pad: int = 12544   # = 49*256
    tch: int = 25088         # gather table chunk rows (< 32768)
    nchunk: int = 4
    win: int = 256           # dst window (psum free size)
    wblk: int = 4            # windows per block (psum-bank limited)
    plan: tuple = ()         # static plan, see make_plan()

    @property
    def trows(self):
        return self.nchunk * self.tch  # 100352 == ncores * shard_pad

    @property
    def nwin(self):
        return self.shard_pad // self.win  # 49

    @property
    def nblk(self):
        return (self.nwin + self.wblk - 1) // self.wblk  # 7

    @property
    def blk_dst(self):
        return self.win * self.wblk  # 2048

    @property
    def nk(self):
        return self.shard_pad // 128

    @property
    def proj_tile(self):
        for ts in (512, 448, 256, 128):
            if self.shard_pad % ts == 0:
                return ts
        raise AssertionError(self.shard_pad)


CFG = Cfg()


# ---------------------------------------------------------------------------
# Static plan: per (block, chunk) slot budget + per tile the list of windows
# it may touch (union over cores/layers), with first/last flags per window.
# ---------------------------------------------------------------------------

def make_plan(edge_sets, cfg):
    """edge_sets: list of (rows, dst) over layers; rows = gather-table row of
    each edge's src, dst = global dst id. Returns (budgets, tiles) where
    budgets[b][k] = padded slots and tiles = tuple of
    (b, k, tile_in_bucket, window, is_first_for_window, is_last_for_window)."""
    nb, nk_ = cfg.nblk, cfg.nchunk
    budgets = np.zeros((nb, nk_), np.int64)
    twin = {}  # (b, k, t) -> set of windows
    for rows, dst in edge_sets:
        ch = rows // cfg.tch
        for c in range(cfg.ncores):
            m = (dst // cfg.shard) == c
            dl = dst[m] - c * cfg.shard          # dst-local [0, 12500)
            kk = ch[m]
            b = dl // cfg.blk_dst
            order = np.lexsort((dl, kk, b))
            bs, ks, ds = b[order], kk[order], dl[order]
            # rank within (b, k)
            gid = bs * nk_ + ks
            first = np.r_[True, gid[1:] != gid[:-1]]
            start = np.maximum.accumulate(np.where(first, np.arange(len(gid)), 0))
            rank = np.arange(len(gid)) - start
            cnt = np.zeros((nb, nk_), np.int64)
            np.add.at(cnt, (bs[first], ks[first]), 0)  # touch
            np.add.at(cnt, (bs, ks), 1)
            budgets = np.maximum(budgets, cnt)
            t = rank // 128
            w = ds // cfg.win
            for key in set(zip(bs.tolist(), ks.tolist(), t.tolist(), w.tolist())):
                twin.setdefault(key[:3], set()).add(key[3])
    budgets = ((budgets + 127) // 128) * 128
    # assemble static tile list in emission order
    tiles = []
    first_seen, entries = set(), []
    for b in range(nb):
        for k in range(nk_):
            for t in range(int(budgets[b][k]) // 128):
                for w in sorted(twin.get((b, k, t), ())):
                    entries.append([b, k, t, w])
    last_idx = {}
    for i, e in enumerate(entries):
        last_idx[e[3]] = i
    seen = set()
    for i, e in enumerate(entries):
        b, k, t, w = e
        tiles.append((b, k, t, w, w not in seen, last_idx[w] == i))
        seen.add(w)
    assert seen == set(range(cfg.nwin)), sorted(set(range(cfg.nwin)) - seen)
    return tuple(tuple(int(v) for v in row) for row in budgets), tuple(tiles)


def _wrap16(arr):
    """[128, n/16] int16 index layout: slot i at [i % 16, i // 16],
    replicated across the 8 groups of 16 partitions (one per Q7 core)."""
    out = arr.reshape(arr.size // 16, 16).T.astype(np.int16)
    return np.ascontiguousarray(np.tile(out, (8, 1)))


def _tilewrap(arr, dtype):
    """[128, n/128] layout: slot i at [i % 128, i // 128] (gather output
    tile layout: consecutive slots go to consecutive partitions)."""
    out = arr.reshape(arr.size // 128, 128).T.astype(dtype)
    return np.ascontiguousarray(out)


def prep_core(rows, dst, core, cfg):
    """Slot assignment for one core/layer. Returns (idx_w, lab_w)."""
    budgets, _ = cfg.plan
    total = sum(sum(r) for r in budgets)
    seg_off = np.zeros((cfg.nblk, cfg.nchunk), np.int64)
    off = 0
    for b in range(cfg.nblk):
        for k in range(cfg.nchunk):
            seg_off[b, k] = off
            off += budgets[b][k]
    m = (dst // cfg.shard) == core
    rows_c = rows[m]
    dl = dst[m] - core * cfg.shard
    kk = rows_c // cfg.tch
    b = dl // cfg.blk_dst
    order = np.lexsort((dl, kk, b))
    bs, ks, ds, rs = b[order], kk[order], dl[order], rows_c[order]
    gid = bs * cfg.nchunk + ks
    first = np.r_[True, gid[1:] != gid[:-1]]
    start = np.maximum.accumulate(np.where(first, np.arange(len(gid)), 0))
    rank = np.arange(len(gid)) - start
    slot = seg_off[bs, ks] + rank
    idx_slots = np.zeros(total, np.int64)
    lab_slots = np.full(total, -1.0, np.float64)
    idx_slots[slot] = rs - ks * cfg.tch
    lab_slots[slot] = ds - bs * cfg.blk_dst   # block-local dst in [0, 2048)
    return _wrap16(idx_slots), _tilewrap(lab_slots, np.float32)


# ---------------------------------------------------------------------------
# Device program
# ---------------------------------------------------------------------------

def build_layer_kernel(tc, outs, ins, cfg):
    from concourse import masks, mybir

    nc = tc.nc
    table, idxs, labs, iota_in, W, bias = ins
    (hout,) = outs
    f32 = mybir.dt.float32
    f16 = mybir.dt.float16
    F = cfg.feat
    NK = cfg.nk
    SP = cfg.shard_pad
    TS = cfg.proj_tile
    WIN = cfg.win
    budgets, tiles = cfg.plan

    with (
        tc.tile_pool(name="const", bufs=1) as constp,
        tc.tile_pool(name="msgp", bufs=12) as msgp,
        tc.tile_pool(name="sp", bufs=8) as spool,
        tc.tile_pool(name="bigs", bufs=1) as bigs,
        tc.tile_pool(name="psW", bufs=5, space="PSUM") as psW,
        tc.tile_pool(name="psP", bufs=2, space="PSUM") as psP,
        tc.tile_pool(name="psH", bufs=1, space="PSUM") as psH,
    ):
        ident = constp.tile([128, 128], f32)
        masks.make_identity(nc, ident[:])
        w_t = constp.tile([F, F], f32)
        nc.sync.dma_start(w_t[:], W)
        b_t = constp.tile([F, 1], f32)
        nc.sync.dma_start(b_t[:], bias)
        iota_t = constp.tile([128, cfg.blk_dst], f16)
        nc.sync.dma_start(iota_t[:], iota_in)

        total = sum(sum(r) for r in budgets)
        idx_t = constp.tile([128, total // 16], mybir.dt.int16)
        nc.sync.dma_start(idx_t[:], idxs)
        lab_t = constp.tile([128, total // 128], f32)
        nc.sync.dma_start(lab_t[:], labs)

        agg_fm = bigs.tile([F, SP], f32)  # feature-major aggregate
        hnm = bigs.tile([128, NK, F], f32)

        # ---- phase A: gather + on-chip matmul aggregation -----------------
        # per (block, chunk) bucket: gather calls of <=512 slots; per
        # 128-slot tile and overlapped window: S = (iota == label) on DVE,
        # psum_w += msg^T @ S on PE.
        msg_tiles = {}   # global tile index -> (tile handle, sub index)
        off = 0
        call_i = 0
        for b in range(cfg.nblk):
            for k in range(cfg.nchunk):
                n_all = budgets[b][k]
                seg = 0
                while seg < n_all:
                    n = min(n_all - seg, 896)
                    msg_t = msgp.tile([128, 7, F], f32, tag="msg")
                    nc.gpsimd.dma_gather(
                        msg_t[:, :n // 128, :],
                        table[k * cfg.tch:(k + 1) * cfg.tch, :],
                        idx_t[:, (off + seg) // 16:(off + seg + n) // 16],
                        num_idxs=n,
                        num_idxs_reg=n,
                        elem_size=F,
                        queue_num=call_i % 4,
                    )
                    # fp16 copy of the messages feeds the (2x faster) fp16
                    # routing matmuls; the scalar engine is otherwise idle
                    msgh = msgp.tile([128, 7, F], f16, tag="msgh")
                    nc.scalar.activation(msgh[:, :n // 128, :],
                                         msg_t[:, :n // 128, :],
                                         mybir.ActivationFunctionType.Copy)
                    for i in range(n // 128):
                        msg_tiles[(b, k, (seg // 128) + i)] = (msgh, i)
                    seg += n
                    call_i += 1
                off += n_all

        seg_off = {}
        off = 0
        for b in range(cfg.nblk):
            for k in range(cfg.nchunk):
                seg_off[(b, k)] = off
                off += budgets[b][k]

        win_psum = {}
        from itertools import groupby
        si = 0
        for (b, k, t), grp in groupby(tiles, key=lambda e: e[:3]):
            grp = list(grp)
            msg_t, sub = msg_tiles[(b, k, t)]
            gt = seg_off[(b, k)] // 128 + t   # global tile index (label col)
            # one is_equal per contiguous window run of this tile; the
            # matmuls read per-window slices of the merged S
            runs = []
            for e in grp:
                if runs and e[3] == runs[-1][-1][3] + 1:
                    runs[-1].append(e)
                else:
                    runs.append([e])
            for run in runs:
                w0 = run[0][3]
                nw = len(run)
                wl0 = w0 - b * cfg.wblk
                s_t = spool.tile([128, cfg.wblk * WIN], f16, name=f"s{si}",
                                 tag="S")
                si += 1
                nc.vector.tensor_scalar(
                    s_t[:, :nw * WIN],
                    iota_t[:, wl0 * WIN:(wl0 + nw) * WIN],
                    lab_t[:, gt:gt + 1], None,
                    op0=mybir.AluOpType.is_equal,
                )
                for j, e in enumerate(run):
                    _eb, _ek, _et, w, is_first, is_last = e
                    self_emit(nc, cfg, w, is_first, is_last, win_psum, psW,
                              psP, psH, msg_t, sub,
                              s_t[:, j * WIN:(j + 1) * WIN], agg_fm, hnm,
                              w_t, b_t, ident, hout, f32, f16, F, WIN, mybir)

        return


def self_emit(nc, cfg, w, is_first, is_last, win_psum, psW, psP, psH, msg_t,
              sub, s_ap, agg_fm, hnm, w_t, b_t, ident, hout, f32, f16, F, WIN,
              mybir):
            if is_first:
                win_psum[w] = psW.tile([F, WIN], f32, name=f"pw{w}", tag="pw")
            nc.tensor.matmul(win_psum[w][:], lhsT=msg_t[:, sub, :], rhs=s_ap,
                             start=is_first, stop=is_last)
            if is_last:
                # fused phase B for this window: copy psum out, project,
                # relu+bias, transpose to node-major, stage for writeout
                sl = slice(w * WIN, (w + 1) * WIN)
                nc.scalar.activation(agg_fm[:, sl], win_psum[w][:],
                                     mybir.ActivationFunctionType.Copy)
                del win_psum[w]
                pp = psP.tile([F, WIN], f32, name=f"pp{w}", tag="pp")
                nc.tensor.matmul(pp[:], lhsT=w_t[:], rhs=agg_fm[:, sl],
                                 start=True, stop=True)
                nc.scalar.activation(agg_fm[:, sl], pp[:],
                                     mybir.ActivationFunctionType.Relu,
                                     bias=b_t[:])
                for kk in range(WIN // 128):
                    k = w * (WIN // 128) + kk
                    ph = psH.tile([128, F], f32, name=f"ph{k}", tag="ph")
                    nc.tensor.transpose(ph[:], agg_fm[:, k * 128:(k + 1) * 128],
                                        ident[:F, :F])
                    nc.vector.tensor_copy(hnm[:, k, :], ph[:])
                nc.sync.dma_start(
                    hout.rearrange("(k p) f -> p k f", p=128)[:, w * 2:w * 2 + 2],
                    hnm[:, w * 2:w * 2 + 2, :])


def build_program(cfg):
    from concourse import bacc, mybir, tile

    f32 = mybir.dt.float32
    i16 = mybir.dt.int16
    budgets, _ = cfg.plan
    total = sum(sum(r) for r in budgets)
    nc = bacc.Bacc("TRN2", target_bir_lowering=False, debug=False,
                   num_devices=cfg.ncores, num_swdge_queues=4)
    table = nc.dram_tensor("table", [cfg.trows, cfg.feat], f32, kind="ExternalInput")
    idxs = nc.dram_tensor("idx", [128, total // 16], i16, kind="ExternalInput")
    labs = nc.dram_tensor("lab", [128, total // 128], f32, kind="ExternalInput")
    iota = nc.dram_tensor("iota", [128, cfg.blk_dst], mybir.dt.float16, kind="ExternalInput")
    W = nc.dram_tensor("W", [cfg.feat, cfg.feat], f32, kind="ExternalInput")
    bias = nc.dram_tensor("bias", [cfg.feat, 1], f32, kind="ExternalInput")
    hout = nc.dram_tensor("hout", [cfg.shard_pad, cfg.feat], f32,
                          kind="ExternalOutput")

    with tile.TileContext(nc) as tc:
        build_layer_kernel(
            tc,
            (hout.ap(),),
            (table.ap(), idxs.ap(), labs.ap(), iota.ap(), W.ap(), bias.ap()),
            cfg,
        )
    nc.compile()
    return nc


_PROGRAMS = {}


def _get_program(cfg):
    if cfg not in _PROGRAMS:
        _PROGRAMS[cfg] = build_program(cfg)
    return _PROGRAMS[cfg]


def _run_layer(nc, cfg, table_pad, preps, W, b, iota_np, **kwargs):
    from concourse.bass_utils import run_bass_kernel_spmd

    in_maps = []
    for c in range(cfg.ncores):
        idx_w, lab_w = preps[c]
        in_maps.append({
            "table": table_pad,
            "idx": idx_w,
            "lab": lab_w,
            "iota": iota_np,
            "W": np.ascontiguousarray(W, np.float32),
            "bias": np.ascontiguousarray(b, np.float32).reshape(cfg.feat, 1),
        })
    return run_bass_kernel_spmd(nc, in_maps, core_ids=list(range(cfg.ncores)),
                                **kwargs)


def kernel(x, src, dst, W1, b1, W2, b2, _cfg=None, _trace=False):
    cfg = _cfg or CFG
    x = np.ascontiguousarray(x, np.float32)
    src = np.asarray(src).astype(np.int64)
    dst = np.asarray(dst).astype(np.int64)

    rows1 = src
    rows2 = (src // cfg.shard) * cfg.shard_pad + (src % cfg.shard)
    if not cfg.plan:
        cfg = replace(cfg, plan=make_plan([(rows1, dst), (rows2, dst)], cfg))
    nc = _get_program(cfg)

    iota_np = np.broadcast_to(
        np.arange(cfg.blk_dst, dtype=np.float16), (128, cfg.blk_dst)
    ).copy()

    table1 = np.zeros((cfg.trows, cfg.feat), np.float32)
    table1[:cfg.n_nodes] = x
    preps1 = [prep_core(rows1, dst, c, cfg) for c in range(cfg.ncores)]
    res1 = _run_layer(nc, cfg, table1, preps1, W1, b1, iota_np,
                      **({"trace": True} if _trace else {}))
    shards1 = [res1.results[c]["hout"] for c in range(cfg.ncores)]

    table2 = np.ascontiguousarray(np.concatenate(shards1, axis=0))
    assert table2.shape[0] == cfg.trows
    preps2 = [prep_core(rows2, dst, c, cfg) for c in range(cfg.ncores)]
    res2 = _run_layer(nc, cfg, table2, preps2, W2, b2, iota_np,
                      **({"trace": True} if _trace else {}))
    shards2 = [res2.results[c]["hout"][:cfg.shard] for c in range(cfg.ncores)]

    out = np.concatenate(shards2, axis=0)
    kernel._last_exec_ns = (
        getattr(res1, "exec_time_ns", None),
        getattr(res2, "exec_time_ns", None),
    )
    return out
